# revision 2
# baseline (speedup 1.0000x reference)
"""Graphormer layer (dense transformer) on 8 Trainium2 NeuronCores.

Strategy: pure data-parallel over batch B=8 (one batch item per core, no
collectives). Per core, the whole layer runs with a transposed-scores
("scoresT") attention layout so softmax denominators come out of the
attention@V matmul itself (augmented-V ones-column trick) and no large
on-device transposes of the [N,N] attention matrices are needed.

Host-side (free, exact algebra):
  - LayerNorm affine (w, b) folded into qkv_w / ffn1_w weights+biases.
  - 1/sqrt(dk) folded into the Q slice of qkv_w.
  - attn_bias pre-transposed to [H, m, n] and pre-exponentiated:
    exp(s + b) = exp(s) * exp(b), with exp(b) computed on host (bf16).

Device layout chain (per core, B-item b):
  x[1024,512] --LN1--> xn --PE transpose--> hT[512,1024]
  qT/kT[f,1024] = qk_w.T @ hT (feature-major), v[1024,520] token-major w/
  ones column per head.  scoresT[m,n] = kT.T-slice @ qT-slice (K=64).
  es = exp(scoresT)*expb;  outT_aug[65,n] += vbig.T @ es (accum over m);
  row 64 = softmax denom;  attnT = outT * bcast(1/denom).
  proj: token-major = attnT.T @ proj_w + residual; LN2; FFN same pattern.

Matmuls in bf16 (fp32 PSUM accumulate); LN/softmax statistics in fp32.
"""

import os
import sys

import numpy as np

sys.path.insert(0, "/opt/trn_rl_repo")

import ml_dtypes

BF16 = ml_dtypes.bfloat16

B, N, D, H, DK = 8, 1024, 512, 8, 64
NT = N // 128  # 8 token tiles
ND = D // 128  # 4 d-tiles
NCORES = 8

_CACHE = {}


def _build_program():
    import concourse.bacc as bacc
    import concourse.tile as tile
    from concourse import mybir
    from concourse.masks import make_identity

    fp32 = mybir.dt.float32
    bf16 = mybir.dt.bfloat16
    Alu = mybir.AluOpType
    Act = mybir.ActivationFunctionType

    nc = bacc.Bacc("TRN2", target_bir_lowering=False, debug=False)

    # ---- DRAM I/O ----
    x_d = nc.dram_tensor("x", [N, D], fp32, kind="ExternalInput")
    expb_d = nc.dram_tensor("expb", [H, N, N], bf16, kind="ExternalInput")
    qkw_d = nc.dram_tensor("qk_w", [D, 2 * D], bf16, kind="ExternalInput")
    vw_d = nc.dram_tensor("v_w", [D, D], bf16, kind="ExternalInput")
    qkb_d = nc.dram_tensor("qk_b", [128, 8], fp32, kind="ExternalInput")
    vb_d = nc.dram_tensor("v_b", [1, D], bf16, kind="ExternalInput")
    pw_d = nc.dram_tensor("proj_w", [D, D], bf16, kind="ExternalInput")
    pb_d = nc.dram_tensor("proj_b", [1, D], bf16, kind="ExternalInput")
    f1w_d = nc.dram_tensor("ffn1_w", [D, 4 * D], bf16, kind="ExternalInput")
    f1b_d = nc.dram_tensor("ffn1_b", [128, 16], fp32, kind="ExternalInput")
    f2w_d = nc.dram_tensor("ffn2_w", [4 * D, D], bf16, kind="ExternalInput")
    f2b_d = nc.dram_tensor("ffn2_b", [1, D], bf16, kind="ExternalInput")
    out_d = nc.dram_tensor("out", [N, D], fp32, kind="ExternalOutput")

    with tile.TileContext(nc) as tc:
        with (
            tc.tile_pool(name="const", bufs=1) as cpool,
            tc.tile_pool(name="resident", bufs=1) as rpool,
        ):
            # ---- constants / weights resident in SBUF ----
            ident = cpool.tile([128, 128], bf16)
            make_identity(nc, ident)
            ones_row = cpool.tile([1, 128], bf16)
            nc.gpsimd.memset(ones_row, 1.0)
            eps_t = cpool.tile([128, 1], fp32)
            nc.gpsimd.memset(eps_t, 1e-5)

            qkw_sb = cpool.tile([128, ND, 2 * D], bf16)
            vw_sb = cpool.tile([128, ND, D], bf16)
            pw_sb = cpool.tile([128, ND, D], bf16)
            f1w_sb = cpool.tile([128, ND, 4 * D], bf16)
            for d in range(ND):
                nc.sync.dma_start(out=qkw_sb[:, d, :], in_=qkw_d[128 * d:128 * (d + 1), :])
                nc.sync.dma_start(out=vw_sb[:, d, :], in_=vw_d[128 * d:128 * (d + 1), :])
                nc.sync.dma_start(out=pw_sb[:, d, :], in_=pw_d[128 * d:128 * (d + 1), :])
                nc.sync.dma_start(out=f1w_sb[:, d, :], in_=f1w_d[128 * d:128 * (d + 1), :])
            qkb_sb = cpool.tile([128, 8], fp32)
            nc.sync.dma_start(out=qkb_sb, in_=qkb_d[:, :])
            f1b_sb = cpool.tile([128, 16], fp32)
            nc.sync.dma_start(out=f1b_sb, in_=f1b_d[:, :])
            vb_row = cpool.tile([1, D], bf16)
            nc.sync.dma_start(out=vb_row, in_=vb_d[:, :])
            pb_row = cpool.tile([1, D], bf16)
            nc.sync.dma_start(out=pb_row, in_=pb_d[:, :])
            f2b_row = cpool.tile([1, D], bf16)
            nc.sync.dma_start(out=f2b_row, in_=f2b_d[:, :])

            # ---- resident activations ----
            x_sb = rpool.tile([128, NT, D], fp32)       # x, later x1 (residual)
            hT_sb = rpool.tile([128, ND, N], bf16)      # hT, later h2T
            qT_sb = rpool.tile([128, ND, N], bf16)
            kT_sb = rpool.tile([128, ND, N], bf16)
            vbig_sb = rpool.tile([128, NT, H * 65], bf16)
            attnT_sb = rpool.tile([128, ND, N], bf16)
            nc.gpsimd.memset(vbig_sb, 1.0)  # ones columns (col 64 of each head)

            def layer_norm_transpose(pool, psum_pool, src_ap, t):
                """standardize src (tokens on partitions) and write transposed
                128-col block t into hT_sb."""
                stats = pool.tile([128, 6], fp32, tag="ln_stats")
                nc.vector.bn_stats(out=stats, in_=src_ap)
                mv = pool.tile([128, 2], fp32, tag="ln_mv")
                nc.vector.bn_aggr(out=mv, in_=stats)
                nc.scalar.activation(
                    out=mv[:, 1:2], in_=mv[:, 1:2], func=Act.Sqrt, bias=eps_t, scale=1.0
                )
                nc.vector.reciprocal(out=mv[:, 1:2], in_=mv[:, 1:2])
                xn = pool.tile([128, D], bf16, tag="ln_xn")
                nc.vector.tensor_scalar(
                    out=xn, in0=src_ap,
                    scalar1=mv[:, 0:1], scalar2=mv[:, 1:2],
                    op0=Alu.subtract, op1=Alu.mult,
                )
                tp = psum_pool.tile([128, ND, 128], bf16, tag="ln_tp")
                for j in range(ND):
                    nc.tensor.transpose(tp[:, j, :], xn[:, 128 * j:128 * (j + 1)], ident)
                nc.vector.tensor_copy(
                    out=hT_sb[:, :, 128 * t:128 * (t + 1)], in_=tp
                )

            # ================= Phase A: load x, LN1, transpose =================
            with (
                tc.tile_pool(name="a_work", bufs=3) as apool,
                tc.tile_pool(name="a_psum", bufs=2, space="PSUM") as apsum,
            ):
                for t in range(NT):
                    nc.sync.dma_start(
                        out=x_sb[:, t, :], in_=x_d[128 * t:128 * (t + 1), :]
                    )
                    layer_norm_transpose(apool, apsum, x_sb[:, t, :], t)

            # ================= Phase B: qT/kT (feature-major), v (token-major) =
            with tc.tile_pool(name="b_psum", bufs=2, space="PSUM") as bpsum:
                for f in range(8):  # 4 q head-pairs then 4 k head-pairs
                    ps = bpsum.tile([128, N], fp32, tag="qk_ps")
                    for d in range(ND):
                        for c in range(2):
                            nc.tensor.matmul(
                                ps[:, 512 * c:512 * (c + 1)],
                                lhsT=qkw_sb[:, d, 128 * f:128 * (f + 1)],
                                rhs=hT_sb[:, d, 512 * c:512 * (c + 1)],
                                start=(d == 0), stop=(d == ND - 1),
                            )
                    dstT = qT_sb if f < 4 else kT_sb
                    nc.vector.tensor_scalar_add(
                        out=dstT[:, f % 4, :], in0=ps, scalar1=qkb_sb[:, f:f + 1]
                    )
                for t in range(NT):
                    ps = bpsum.tile([128, D], fp32, tag="v_ps")
                    for d in range(ND):
                        nc.tensor.matmul(
                            ps,
                            lhsT=hT_sb[:, d, 128 * t:128 * (t + 1)],
                            rhs=vw_sb[:, d, :],
                            start=(d == 0), stop=False,
                        )
                    nc.tensor.matmul(
                        ps, lhsT=ones_row[:, 0:128], rhs=vb_row,
                        start=False, stop=True,
                    )
                    nc.vector.tensor_copy(
                        out=vbig_sb[:, t, :].rearrange("p (h c) -> p h c", c=65)[:, :, 0:64],
                        in_=ps.rearrange("p (h c) -> p h c", c=64),
                    )

            # ================= Phase C: attention per head ====================
            with (
                tc.tile_pool(name="c_work", bufs=3) as cwork,
                tc.tile_pool(name="c_psum", bufs=2, space="PSUM") as cpsum,
            ):
                for h in range(H):
                    f, off = h // 2, (h % 2) * 64
                    op = cpsum.tile([65, N], fp32, tag="out_ps")
                    for m in range(NT):
                        bt = cwork.tile([128, N], bf16, tag="expb_t")
                        nc.sync.dma_start(
                            out=bt, in_=expb_d[h, 128 * m:128 * (m + 1), :]
                        )
                        sc = cpsum.tile([128, N], fp32, tag="sc_ps")
                        for c in range(2):
                            nc.tensor.matmul(
                                sc[:, 512 * c:512 * (c + 1)],
                                lhsT=kT_sb[off:off + 64, f, 128 * m:128 * (m + 1)],
                                rhs=qT_sb[off:off + 64, f, 512 * c:512 * (c + 1)],
                                start=True, stop=True,
                            )
                        es = cwork.tile([128, N], bf16, tag="es_t")
                        nc.scalar.activation(out=es, in_=sc, func=Act.Exp)
                        nc.vector.tensor_tensor(out=es, in0=es, in1=bt, op=Alu.mult)
                        for c in range(2):
                            nc.tensor.matmul(
                                op[:, 512 * c:512 * (c + 1)],
                                lhsT=vbig_sb[:, m, 65 * h:65 * (h + 1)],
                                rhs=es[:, 512 * c:512 * (c + 1)],
                                start=(m == 0), stop=(m == NT - 1),
                            )
                    # normalize: attnT = outT[0:64] * bcast(1 / denom-row)
                    den_b = cwork.tile([1, N], bf16, tag="den_b")
                    nc.scalar.activation(out=den_b, in_=op[64:65, :], func=Act.Copy)
                    rb = cpsum.tile([64, N], fp32, tag="sc_ps")
                    for c in range(2):
                        nc.tensor.matmul(
                            rb[:, 512 * c:512 * (c + 1)],
                            lhsT=ones_row[:, 0:64],
                            rhs=den_b[:, 512 * c:512 * (c + 1)],
                            start=True, stop=True,
                        )
                    rbs = cwork.tile([64, N], fp32, tag="rbs_t")
                    nc.vector.reciprocal(out=rbs, in_=rb)
                    nc.vector.tensor_tensor(
                        out=attnT_sb[off:off + 64, f, :],
                        in0=op[0:64, :], in1=rbs, op=Alu.mult,
                    )

            # ================= Phase D: proj + residual, LN2 ==================
            with (
                tc.tile_pool(name="d_work", bufs=3) as dpool,
                tc.tile_pool(name="d_psum", bufs=2, space="PSUM") as dpsum,
            ):
                for t in range(NT):
                    pr = dpsum.tile([128, D], fp32, tag="pr_ps")
                    for p in range(ND):
                        nc.tensor.matmul(
                            pr,
                            lhsT=attnT_sb[:, p, 128 * t:128 * (t + 1)],
                            rhs=pw_sb[:, p, :],
                            start=(p == 0), stop=False,
                        )
                    nc.tensor.matmul(
                        pr, lhsT=ones_row[:, 0:128], rhs=pb_row,
                        start=False, stop=True,
                    )
                    nc.vector.tensor_tensor(
                        out=x_sb[:, t, :], in0=pr, in1=x_sb[:, t, :], op=Alu.add
                    )
                    layer_norm_transpose(dpool, dpsum, x_sb[:, t, :], t)

            # ================= Phase E: FFN + residual, store =================
            with (
                tc.tile_pool(name="e_work", bufs=3) as ework,
                tc.tile_pool(name="e_psum", bufs=1, space="PSUM") as epsum,
                tc.tile_pool(name="e_psum2", bufs=2, space="PSUM") as epsum2,
            ):
                for g in range(2):
                    ops = []
                    for q in range(4):
                        op_t = epsum.tile([128, D], fp32, tag=f"eout{q}")
                        ops.append(op_t)
                    for fi in range(16):
                        fh = epsum2.tile([128, D], fp32, tag="fh_ps")
                        for d in range(ND):
                            nc.tensor.matmul(
                                fh,
                                lhsT=f1w_sb[:, d, 128 * fi:128 * (fi + 1)],
                                rhs=hT_sb[:, d, 512 * g:512 * (g + 1)],
                                start=(d == 0), stop=(d == ND - 1),
                            )
                        rl = ework.tile([128, D], bf16, tag="rl_t")
                        nc.scalar.activation(
                            out=rl, in_=fh, func=Act.Relu,
                            bias=f1b_sb[:, fi:fi + 1], scale=1.0,
                        )
                        w2 = ework.tile([128, D], bf16, tag="w2_t")
                        nc.sync.dma_start(
                            out=w2, in_=f2w_d[128 * fi:128 * (fi + 1), :]
                        )
                        for q in range(4):
                            nc.tensor.matmul(
                                ops[q],
                                lhsT=rl[:, 128 * q:128 * (q + 1)],
                                rhs=w2,
                                start=(fi == 0), stop=False,
                            )
                    for q in range(4):
                        t = 4 * g + q
                        nc.tensor.matmul(
                            ops[q], lhsT=ones_row[:, 0:128], rhs=f2b_row,
                            start=False, stop=True,
                        )
                        ot = ework.tile([128, D], fp32, tag="ot_t")
                        nc.vector.tensor_tensor(
                            out=ot, in0=ops[q], in1=x_sb[:, t, :], op=Alu.add
                        )
                        nc.sync.dma_start(
                            out=out_d[128 * t:128 * (t + 1), :], in_=ot
                        )

    nc.compile()
    return nc


def get_program():
    if "nc" not in _CACHE:
        _CACHE["nc"] = _build_program()
    return _CACHE["nc"]


def prep_inputs(inputs):
    """Host-side: fold LN affines + 1/sqrt(dk) into weights, pre-exp the
    attention bias, build per-core input maps."""
    f32 = np.float32
    x = np.asarray(inputs["x"], f32)
    attn_bias = np.asarray(inputs["attn_bias"], f32)
    qkv_w = np.asarray(inputs["qkv_w"], f32)
    qkv_b = np.asarray(inputs["qkv_b"], f32)
    proj_w = np.asarray(inputs["proj_w"], f32)
    proj_b = np.asarray(inputs["proj_b"], f32)
    ffn1_w = np.asarray(inputs["ffn1_w"], f32)
    ffn1_b = np.asarray(inputs["ffn1_b"], f32)
    ffn2_w = np.asarray(inputs["ffn2_w"], f32)
    ffn2_b = np.asarray(inputs["ffn2_b"], f32)
    ln1_w = np.asarray(inputs["ln1_w"], f32)
    ln1_b = np.asarray(inputs["ln1_b"], f32)
    ln2_w = np.asarray(inputs["ln2_w"], f32)
    ln2_b = np.asarray(inputs["ln2_b"], f32)

    qkv_w_eff = qkv_w * ln1_w[:, None]
    qkv_b_eff = qkv_b + ln1_b @ qkv_w
    scale = 1.0 / np.sqrt(DK).astype(f32)
    qkv_w_eff[:, :D] *= scale
    qkv_b_eff = qkv_b_eff.copy()
    qkv_b_eff[:D] *= scale

    ffn1_w_eff = ffn1_w * ln2_w[:, None]
    ffn1_b_eff = ffn1_b + ln2_b @ ffn1_w

    # exp(bias), transposed to [B, H, m, n]
    expb = np.exp(attn_bias.transpose(0, 1, 3, 2)).astype(BF16)

    shared = {
        "qk_w": np.ascontiguousarray(qkv_w_eff[:, : 2 * D]).astype(BF16),
        "v_w": np.ascontiguousarray(qkv_w_eff[:, 2 * D:]).astype(BF16),
        "qk_b": np.ascontiguousarray(qkv_b_eff[: 2 * D].reshape(8, 128).T),
        "v_b": qkv_b_eff[None, 2 * D:].astype(BF16),
        "proj_w": proj_w.astype(BF16),
        "proj_b": proj_b[None].astype(BF16),
        "ffn1_w": ffn1_w_eff.astype(BF16),
        "ffn1_b": np.ascontiguousarray(ffn1_b_eff.reshape(16, 128).T),
        "ffn2_w": ffn2_w.astype(BF16),
        "ffn2_b": ffn2_b[None].astype(BF16),
    }
    in_maps = [
        {"x": np.ascontiguousarray(x[b]), "expb": np.ascontiguousarray(expb[b]), **shared}
        for b in range(B)
    ]
    return in_maps


def run(in_maps, trace=False, **kw):
    from concourse.bass_utils import run_bass_kernel_spmd

    nc = get_program()
    return run_bass_kernel_spmd(
        nc, in_maps, core_ids=list(range(NCORES)), trace=trace, **kw
    )


def kernel(**inputs) -> np.ndarray:
    in_maps = prep_inputs(inputs)
    res = run(in_maps, trace=False)
    return np.stack([np.asarray(r["out"], np.float32) for r in res.results], axis=0)


if __name__ == "__main__":
    # quick smoke: random inputs through the device path
    rng = np.random.default_rng(0)
    ins = {
        "x": rng.standard_normal((B, N, D), dtype=np.float32),
        "attn_bias": rng.standard_normal((B, H, N, N), dtype=np.float32),
        "qkv_w": rng.standard_normal((D, 3 * D), dtype=np.float32) * 0.02,
        "qkv_b": np.zeros(3 * D, np.float32),
        "proj_w": rng.standard_normal((D, D), dtype=np.float32) * 0.02,
        "proj_b": np.zeros(D, np.float32),
        "ffn1_w": rng.standard_normal((D, 4 * D), dtype=np.float32) * 0.02,
        "ffn1_b": np.zeros(4 * D, np.float32),
        "ffn2_w": rng.standard_normal((4 * D, D), dtype=np.float32) * 0.02,
        "ffn2_b": np.zeros(D, np.float32),
        "ln1_w": np.ones(D, np.float32),
        "ln1_b": np.zeros(D, np.float32),
        "ln2_w": np.ones(D, np.float32),
        "ln2_b": np.zeros(D, np.float32),
    }
    out = kernel(**ins)
    print("out", out.shape, out.dtype, float(np.abs(out).max()))
